# revision 3
# baseline (speedup 1.0000x reference)
"""3-layer GCN (PyG GCNConv-style) on 8 Trainium2 NeuronCores.

Distribution: 1-D node partition (2048 nodes per core). Per core:
  - GEMM1: h1T[36,2048] = W1^T @ x[rows_c]^T, streaming the host-transposed
    x shard (fp32, PE, PSUM accumulation over 128 K-chunks).
  - Per layer: messages g = dis * h (dis = 1/sqrt(deg)) are AllGathered
    (fp16) and aggregated with a *dense* per-core adjacency block
    At[16384, 2048] in fp8 (entries = exact small edge multiplicities;
    the symmetric norm is folded out as dis pre/post scaling), via PE
    matmuls accumulating aggT[36, 2048] in PSUM.
  - Small GEMMs (W2, W3) in fp32, softmax over the 16 classes at the end.
"""
import numpy as np
import concourse.bacc as bacc
import concourse.mybir as mybir
import concourse.tile as tile
from concourse.bass_utils import run_bass_kernel_spmd

N = 16384
E = 524288
H = 36
C = 16
NCORES = 8
ND = N // NCORES          # 2048 nodes per core
KC = N // 128             # 128 contraction chunks of 128 nodes
RB = ND // 128            # 16 row-blocks per core
FP8 = mybir.dt.np(mybir.dt.float8e4)

_PROGRAM = None


def _build_program():
    nc = bacc.Bacc(None)
    f32, f16, fp8 = mybir.dt.float32, mybir.dt.float16, mybir.dt.float8e4

    xT_d = nc.dram_tensor("xT", [N, ND], f32, kind="ExternalInput")
    W1c_d = nc.dram_tensor("W1c", [128, KC * H], f32, kind="ExternalInput")
    A_d = nc.dram_tensor("A", [KC, 128, ND], fp8, kind="ExternalInput")
    disr_d = nc.dram_tensor("disr", [H, ND], f32, kind="ExternalInput")
    W2_d = nc.dram_tensor("W2", [H, H], f32, kind="ExternalInput")
    W3_d = nc.dram_tensor("W3", [H, C], f32, kind="ExternalInput")
    b1_d = nc.dram_tensor("b1", [H, 1], f32, kind="ExternalInput")
    b2_d = nc.dram_tensor("b2", [H, 1], f32, kind="ExternalInput")
    b3_d = nc.dram_tensor("b3", [C, 1], f32, kind="ExternalInput")
    I_d = nc.dram_tensor("ident", [128, 128], f32, kind="ExternalInput")
    out_d = nc.dram_tensor("out", [ND, C], f32, kind="ExternalOutput")

    cc_in = [nc.dram_tensor(f"cc{l}_in", [ND, H if l < 2 else C], f16)
             for l in range(3)]
    cc_out = [nc.dram_tensor(f"cc{l}_out", [N, H if l < 2 else C], f16,
                             addr_space="Shared")
              for l in range(3)]
    groups = [list(range(NCORES))]

    with tile.TileContext(nc) as tc:
        with (
            tc.tile_pool(name="const", bufs=1) as constp,
            tc.tile_pool(name="xs", bufs=3) as xpool,
            tc.tile_pool(name="apool", bufs=4) as apool,
            tc.tile_pool(name="gt", bufs=1) as gtp,
            tc.tile_pool(name="work", bufs=1) as work,
            tc.tile_pool(name="psb", bufs=1, space="PSUM") as psb,
            tc.tile_pool(name="pst", bufs=2, space="PSUM") as pst,
        ):
            W1c = constp.tile([128, KC, H], f32)
            disr = constp.tile([H, ND], f32)
            W2t = constp.tile([H, H], f32)
            W3t = constp.tile([H, C], f32)
            b1t = constp.tile([H, 1], f32)
            b2t = constp.tile([H, 1], f32)
            b3t = constp.tile([C, 1], f32)
            ident = constp.tile([128, 128], f32)
            nc.sync.dma_start(W1c[:], W1c_d[:].rearrange("p (c f) -> p c f", f=H))
            nc.sync.dma_start(disr[:], disr_d[:])
            nc.sync.dma_start(W2t[:], W2_d[:])
            nc.sync.dma_start(W3t[:], W3_d[:])
            nc.sync.dma_start(b1t[:], b1_d[:])
            nc.sync.dma_start(b2t[:], b2_d[:])
            nc.sync.dma_start(b3t[:], b3_d[:])
            nc.sync.dma_start(ident[:], I_d[:])

            # ---- GEMM1: h1T[36, 2048] += W1[kc]^T @ xT[kc] ----
            hT = psb.tile([H, ND], f32, tag="big")
            for kc in range(KC):
                xt = xpool.tile([128, ND], f32, tag="x")
                nc.sync.dma_start(xt[:], xT_d[kc * 128:(kc + 1) * 128, :])
                for q in range(4):
                    nc.tensor.matmul(
                        hT[:, q * 512:(q + 1) * 512],
                        W1c[:, kc, :],
                        xt[:, q * 512:(q + 1) * 512],
                        start=(kc == 0),
                        stop=(kc == KC - 1),
                    )

            for layer in range(3):
                F = H if layer < 2 else C
                # ---- prescale by dis (own rows), share, gather table ----
                gT = work.tile([H, ND], f32, tag="gT")
                nc.vector.tensor_tensor(
                    gT[0:F, :], hT[0:F, :], disr[0:F, :], mybir.AluOpType.mult
                )
                gown = work.tile([128, RB, F], f16, tag="gown")
                for rb in range(RB):
                    tp = pst.tile([128, H], f32, tag="tp")
                    nc.tensor.transpose(
                        tp[:, 0:F],
                        gT[0:F, rb * 128:(rb + 1) * 128],
                        ident[0:F, 0:F],
                    )
                    nc.vector.tensor_copy(gown[:, rb, :], tp[:, 0:F])
                nc.sync.dma_start(
                    cc_in[layer][:].rearrange("(b p) f -> p b f", p=128), gown[:]
                )
                nc.gpsimd.collective_compute(
                    "AllGather",
                    mybir.AluOpType.bypass,
                    replica_groups=groups,
                    ins=[cc_in[layer][:]],
                    outs=[cc_out[layer][:]],
                )
                g_t = gtp.tile([128, KC, F], f16, tag="g")
                nc.sync.dma_start(
                    g_t[:], cc_out[layer][:].rearrange("(p c) f -> p c f", p=128)
                )

                # ---- dense aggregation: aggT[F, 2048] += g[cch]^T @ A[cch] ----
                aggT = psb.tile([H, ND], f32, tag="big")
                for cch in range(KC):
                    a_t = apool.tile([128, ND], fp8, tag="a")
                    nc.sync.dma_start(a_t[:], A_d[cch, :, :])
                    for q in range(4):
                        nc.tensor.matmul(
                            aggT[0:F, q * 512:(q + 1) * 512],
                            g_t[:, cch, :],
                            a_t[:, q * 512:(q + 1) * 512],
                            start=(cch == 0),
                            stop=(cch == KC - 1),
                        )

                if layer < 2:
                    # in_{l+1}T = relu(dis*aggT + b)
                    tmp = work.tile([H, ND], f32, tag="tmp")
                    nc.vector.tensor_tensor(
                        tmp[:], aggT[0:H, :], disr[:], mybir.AluOpType.mult
                    )
                    inT = work.tile([H, ND], f32, tag="inT")
                    nc.scalar.activation(
                        inT[:], tmp[:], mybir.ActivationFunctionType.Relu,
                        bias=b1t[:] if layer == 0 else b2t[:],
                    )
                    # next-layer GEMM: hT = W^T @ inT  (K = 36)
                    Wt = W2t if layer == 0 else W3t
                    Fn = H if layer == 0 else C
                    hT = psb.tile([H, ND], f32, tag="big")
                    for q in range(4):
                        nc.tensor.matmul(
                            hT[0:Fn, q * 512:(q + 1) * 512],
                            Wt[:],
                            inT[:, q * 512:(q + 1) * 512],
                            start=True,
                            stop=True,
                        )
                else:
                    # logitsT = dis*aggT + b3 ; then softmax over classes
                    tmp = work.tile([H, ND], f32, tag="tmp")
                    nc.vector.tensor_tensor(
                        tmp[0:C, :], aggT[0:C, :], disr[0:C, :],
                        mybir.AluOpType.mult,
                    )
                    logT = work.tile([C, ND], f32, tag="logT")
                    nc.vector.tensor_scalar(
                        logT[:], tmp[0:C, :], b3t[:], None, mybir.AluOpType.add
                    )
                    # transpose to natural [2048, 16]
                    onat = work.tile([128, RB, C], f32, tag="onat")
                    for rb in range(RB):
                        tp = pst.tile([128, H], f32, tag="tp")
                        nc.tensor.transpose(
                            tp[:, 0:C],
                            logT[:, rb * 128:(rb + 1) * 128],
                            ident[0:C, 0:C],
                        )
                        nc.vector.tensor_copy(onat[:, rb, :], tp[:, 0:C])
                    # softmax along the class (free) dim
                    negmax = work.tile([128, RB], f32, tag="negmax")
                    nc.vector.tensor_reduce(
                        negmax[:], onat[:], axis=mybir.AxisListType.X,
                        op=mybir.AluOpType.max, negate=True,
                    )
                    expv = work.tile([128, RB, C], f32, tag="expv")
                    ssum = work.tile([128, RB], f32, tag="ssum")
                    for rb in range(RB):
                        nc.scalar.activation(
                            expv[:, rb, :], onat[:, rb, :],
                            mybir.ActivationFunctionType.Exp,
                            bias=negmax[:, rb:rb + 1],
                            accum_out=ssum[:, rb:rb + 1],
                        )
                    rsum = work.tile([128, RB], f32, tag="rsum")
                    nc.vector.reciprocal(rsum[:], ssum[:])
                    prob = work.tile([128, RB, C], f32, tag="prob")
                    for rb in range(RB):
                        nc.vector.tensor_scalar(
                            prob[:, rb, :], expv[:, rb, :],
                            rsum[:, rb:rb + 1], None, mybir.AluOpType.mult,
                        )
                    nc.sync.dma_start(
                        out_d[:].rearrange("(b p) f -> p b f", p=128), prob[:]
                    )

    nc.finalize()
    return nc


def _get_program():
    global _PROGRAM
    if _PROGRAM is None:
        _PROGRAM = _build_program()
    return _PROGRAM


def kernel(x, edge_index, W1, b1, W2, b2, W3, b3, _profile=False):
    x = np.asarray(x, dtype=np.float32)
    edge_index = np.asarray(edge_index)
    W1 = np.asarray(W1, dtype=np.float32)
    W2 = np.asarray(W2, dtype=np.float32)
    W3 = np.asarray(W3, dtype=np.float32)
    b1 = np.asarray(b1, dtype=np.float32)
    b2 = np.asarray(b2, dtype=np.float32)
    b3 = np.asarray(b3, dtype=np.float32)

    # ---- graph preprocessing (host) ----
    loop = np.arange(N, dtype=np.int64)
    src = np.concatenate([edge_index[0].astype(np.int64), loop])
    dst = np.concatenate([edge_index[1].astype(np.int64), loop])
    deg = np.bincount(dst, minlength=N).astype(np.float32)
    dis = (1.0 / np.sqrt(np.maximum(deg, np.float32(1.0)))).astype(np.float32)

    order = np.argsort(dst)
    src_s, dst_s = src[order], dst[order]
    core_of = dst_s // ND
    bounds = np.searchsorted(core_of, np.arange(NCORES + 1))

    W1c = np.ascontiguousarray(
        W1.reshape(KC, 128, H).transpose(1, 0, 2)
    ).reshape(128, KC * H)
    ident = np.eye(128, dtype=np.float32)

    in_maps = []
    for c in range(NCORES):
        lo, hi = bounds[c], bounds[c + 1]
        A = np.zeros((N, ND), dtype=np.float32)
        np.add.at(A, (src_s[lo:hi], dst_s[lo:hi] - c * ND), np.float32(1.0))
        A8 = np.ascontiguousarray(
            A.astype(FP8).reshape(128, KC, ND).transpose(1, 0, 2)
        )
        xT = np.ascontiguousarray(x[c * ND:(c + 1) * ND, :].T)
        disr = np.ascontiguousarray(
            np.broadcast_to(dis[c * ND:(c + 1) * ND][None, :], (H, ND))
        )
        in_maps.append({
            "xT": xT,
            "W1c": W1c,
            "A": A8,
            "disr": disr,
            "W2": W2,
            "W3": W3,
            "b1": b1.reshape(H, 1),
            "b2": b2.reshape(H, 1),
            "b3": b3.reshape(C, 1),
            "ident": ident,
        })

    nc = _get_program()
    res = run_bass_kernel_spmd(nc, in_maps, list(range(NCORES)),
                               trace=bool(_profile))
    out = np.concatenate([res.results[c]["out"] for c in range(NCORES)], axis=0)
    if _profile:
        return out, res.exec_time_ns
    return out
